# revision 1
# baseline (speedup 1.0000x reference)
"""Multi-head self-attention (B=1, S=2048, E=1024, H=16, D=64) on 8 NeuronCores.

Tensor-parallel by head: core c owns heads {2c, 2c+1}. Each core computes
  qT/kT = (w_q/k^T @ qkv^T + b)        [128, S]   (2 heads x 64 stacked)
  v     = qkv @ w_v + b_v              [S, 128]
  scoresT_h = k_h @ q_h^T              [S(t), S(s)] per head (K=64 matmuls,
                                        both heads concurrent via PE row groups)
  expT_h = exp(scoresT / 8)            (no max-subtraction: scores ~ N(0,1))
  [outT_h; Z_h] = [v_h | 1]^T @ expT_h [65, S]    (ones column -> softmax denom)
  attn_outT = outT_h * (1/Z_h)         [128, S]
  partial = attn_outT^T @ w_out_rows   [S, E]
Host sums the 8 partials and adds b_out.
"""

import os
import sys

import numpy as np

try:
    import concourse.bass as bass  # noqa: F401
except ImportError:
    sys.path.insert(0, "/opt/trn_rl_repo")

import ml_dtypes

import concourse.bass as bass
import concourse.mybir as mybir
import concourse.tile as tile
from concourse import bacc, bass_utils

S = 2048
E = 1024
H = 16
D = 64
NCORE = 8
HC = H // NCORE          # heads per core = 2
J = HC * D               # local feature width = 128
KE = E // 128            # contraction tiles for in_proj = 8
NT = S // 128            # 128-row tiles of the sequence = 16
CH = 512                 # free-dim chunk (one PSUM bank)
NCH = S // CH            # s-chunks = 4
SCALE = 1.0 / np.sqrt(D)

MM_DT = mybir.dt.bfloat16
MM_NP = ml_dtypes.bfloat16

_cached = None


def _build():
    f32 = mybir.dt.float32
    nc = bacc.Bacc("TRN2", target_bir_lowering=False, num_swdge_queues=4)

    d_qkvT = nc.dram_tensor("qkvT", (E, S), MM_DT, kind="ExternalInput")
    d_wq = nc.dram_tensor("wq", (E, J), MM_DT, kind="ExternalInput")
    d_wk = nc.dram_tensor("wk", (E, J), MM_DT, kind="ExternalInput")
    d_wv = nc.dram_tensor("wv", (E, J), MM_DT, kind="ExternalInput")
    d_bq = nc.dram_tensor("bq", (J, 1), f32, kind="ExternalInput")
    d_bk = nc.dram_tensor("bk", (J, 1), f32, kind="ExternalInput")
    d_bv = nc.dram_tensor("bv", (128, J), f32, kind="ExternalInput")
    d_wout = nc.dram_tensor("wout", (J, E), MM_DT, kind="ExternalInput")
    f16 = mybir.dt.float16
    d_out = nc.dram_tensor("partial", (S, E), f16, kind="ExternalOutput")

    NP = 2            # s-chunks fused per exp op
    W = NP * CH       # 1024

    with tile.TileContext(nc) as tc:
        with (
            tc.tile_pool(name="persist", bufs=1) as persist,
            tc.tile_pool(name="outp", bufs=3) as outp,
            tc.tile_pool(name="small", bufs=2) as small,
            # single PSUM pool: 4 slots x [128, 1024] = all 8 banks; every
            # phase (in_proj, scores, AV accumulators, out_proj) shares them
            tc.tile_pool(name="ps_big", bufs=4, space="PSUM") as ps_big,
        ):
            # ---- persistent SBUF ----
            sb_wq = persist.tile([128, KE, J], MM_DT)
            sb_wk = persist.tile([128, KE, J], MM_DT)
            sb_wv = persist.tile([128, KE, J], MM_DT)
            sb_bq = persist.tile([J, 1], f32)
            sb_bk = persist.tile([J, 1], f32)
            sb_bv = persist.tile([128, J], f32)
            sb_wout = persist.tile([J, E], MM_DT)
            sb_qT = persist.tile([J, S], MM_DT)
            sb_kT = persist.tile([J, S], MM_DT)
            # v augmented per head with a 64-wide ones block: the AV matmul
            # then yields Z replicated on partitions 64..127 (broadcast free)
            sb_v = persist.tile([128, NT, HC * 2 * D], MM_DT)
            sb_attnT = persist.tile([J, S], MM_DT)

            nc.sync.dma_start(
                out=sb_wk[:], in_=d_wk.rearrange("(k p) m -> p k m", p=128)
            )
            nc.scalar.dma_start(
                out=sb_wq[:], in_=d_wq.rearrange("(k p) m -> p k m", p=128)
            )
            nc.gpsimd.dma_start(
                out=sb_wv[:], in_=d_wv.rearrange("(k p) m -> p k m", p=128)
            )
            nc.gpsimd.dma_start(out=sb_bq[:], in_=d_bq[:])
            nc.gpsimd.dma_start(out=sb_bk[:], in_=d_bk[:])
            nc.gpsimd.dma_start(out=sb_bv[:], in_=d_bv[:])
            nc.gpsimd.dma_start(out=sb_wout[:], in_=d_wout[:])
            nc.vector.memset(sb_v[:, :, D : 2 * D], 1.0)
            nc.vector.memset(sb_v[:, :, 3 * D :], 1.0)

            def proj_qk(sb_w, sb_b, sb_dst, c, sb_qkvT):
                ps = ps_big.tile([128, W], f32, tag="big", name="ps_qk")
                for k in range(KE):
                    nc.tensor.matmul(
                        ps[:, :CH],
                        sb_w[:, k, :],
                        sb_qkvT[:, k, c * CH : (c + 1) * CH],
                        start=(k == 0),
                        stop=(k == KE - 1),
                    )
                nc.vector.tensor_scalar_add(
                    sb_dst[:, c * CH : (c + 1) * CH], ps[:, :CH], sb_b[:]
                )

            def proj_v(t, sb_qkvT):
                ps = ps_big.tile([128, W], f32, tag="big", name="ps_v")
                for k in range(KE):
                    nc.tensor.matmul(
                        ps[:, :J],
                        sb_qkvT[:, k, t * 128 : (t + 1) * 128],
                        sb_wv[:, k, :],
                        start=(k == 0),
                        stop=(k == KE - 1),
                    )
                for h in range(HC):
                    nc.vector.tensor_add(
                        sb_v[:, t, h * 2 * D : h * 2 * D + D],
                        ps[:, h * D : (h + 1) * D],
                        sb_bv[:, h * D : (h + 1) * D],
                    )

            def scores_t(ex, p, t):
                # scores^T for s-chunks {2p, 2p+1}, both heads, t-tile t.
                # Head MMs alternate PE row groups (K=64, base partition 0/64)
                # so consecutive matmuls run concurrently on the array.
                tiles = [
                    ps_big.tile([128, W], f32, tag="big", name=f"ps_s{h}")
                    for h in range(HC)
                ]
                for i in range(NP):
                    for h in range(HC):
                        hd = slice(h * D, (h + 1) * D)
                        nc.tensor.matmul(
                            tiles[h][:, i * CH : (i + 1) * CH],
                            sb_kT[hd, t * 128 : (t + 1) * 128],
                            sb_qT[hd, (p * W + i * CH) : (p * W + (i + 1) * CH)],
                            start=True,
                            stop=True,
                        )
                for h in range(HC):
                    nc.scalar.activation(
                        ex[h][:, t, :],
                        tiles[h][:],
                        mybir.ActivationFunctionType.Exp,
                        scale=float(SCALE),
                    )

            def av_alloc():
                # one [128, W] accumulator per head; chunk ci of the pair in
                # free-dim half ci. rows 0:D = out^T, rows D:2D = Z (bcast)
                return [
                    ps_big.tile([128, W], f32, tag="big", name=f"ps_av{h}")
                    for h in range(HC)
                ]

            def av_t(avp, ex_pair, t, cis=tuple(range(NP))):
                for h in range(HC):
                    for ci in cis:
                        nc.tensor.matmul(
                            avp[h][:, ci * CH : (ci + 1) * CH],
                            sb_v[:, t, h * 2 * D : (h + 1) * 2 * D],
                            ex_pair[h][:, t, ci * CH : (ci + 1) * CH],
                            start=(t == 0),
                            stop=(t == NT - 1),
                        )

            def act_recip(out_ap, in_ap):
                # ScalarE spline reciprocal: ~1e-5 rel err on Z in [30, 6e3]
                # (measured on HW) -- plenty for the bf16 pipeline, 0.72us vs
                # 3.3us for the exact DVE reciprocal
                eng = nc.scalar
                inst = mybir.InstActivation(
                    name=nc.get_next_instruction_name(),
                    func=mybir.ActivationFunctionType.Reciprocal,
                    ins=[
                        eng.lower_ap(in_ap),
                        mybir.ImmediateValue(dtype=f32, value=0.0),
                        mybir.ImmediateValue(dtype=f32, value=1.0),
                        mybir.ImmediateValue(dtype=f32, value=0.0),
                    ],
                    outs=[eng.lower_ap(out_ap)],
                )
                eng.add_instruction(inst)

            def norm_h(avp, p, ci, h):
                c = p * NP + ci
                s_sl = slice(c * CH, (c + 1) * CH)
                hd = slice(h * D, (h + 1) * D)
                rbc = small.tile([D, CH], f32, tag="rbc", name="rbc")
                act_recip(rbc[:], avp[h][D : 2 * D, ci * CH : (ci + 1) * CH])
                nc.vector.tensor_mul(
                    sb_attnT[hd, s_sl],
                    avp[h][:D, ci * CH : (ci + 1) * CH],
                    rbc[:],
                )

            def norm(avp, p, ci):
                for h in range(HC):
                    norm_h(avp, p, ci, h)

            def out_proj_chunk(c, act_evict=False):
                for st in range(CH // 128):
                    t = c * (CH // 128) + st
                    ps_p = ps_big.tile([128, W], f32, tag="big", name="ps_p")
                    for ec in range(E // CH):
                        nc.tensor.matmul(
                            ps_p[:, ec * CH : (ec + 1) * CH],
                            sb_attnT[:, t * 128 : (t + 1) * 128],
                            sb_wout[:, ec * CH : (ec + 1) * CH],
                            start=True,
                            stop=True,
                        )
                    sb_out = outp.tile([128, E], f16, tag="out", name="sb_out")
                    if act_evict and t % 2 == 1:
                        nc.scalar.copy(sb_out[:], ps_p[:])
                    else:
                        nc.vector.tensor_copy(sb_out[:], ps_p[:])
                    nc.sync.dma_start(
                        out=d_out[t * 128 : (t + 1) * 128, :], in_=sb_out[:]
                    )

            with tc.tile_pool(name="exppA", bufs=1) as exppA:
                exA = [
                    exppA.tile([128, NT, W], MM_DT, tag=f"e{h}", name=f"eA{h}")
                    for h in range(HC)
                ]
                with tc.tile_pool(name="qkvp", bufs=1) as qkvp:
                    sb_qkvT = qkvp.tile([128, KE, S], MM_DT)
                    warm = ps_big.tile([128, W], f32, tag="big", name="warm")
                    for _ in range(24):
                        nc.tensor.matmul(
                            warm[:, :CH],
                            sb_wk[:, 0, :],
                            sb_wk.rearrange("p k m -> p (k m)")[:, :CH],
                            start=True,
                            stop=True,
                        )
                    qengs = [nc.sync, nc.scalar, nc.gpsimd, nc.gpsimd]
                    for k in range(KE):
                        qengs[k % 4].dma_start(
                            out=sb_qkvT[:, k, :],
                            in_=d_qkvT[k * 128 : (k + 1) * 128, :],
                        )
                    # k-outer warm-up: the first three projections issue one
                    # matmul per arriving qkv k-tile, so the PE starts as soon
                    # as DMA delivers tile 0 and scores can begin ~2 chunks in
                    first = [
                        (sb_wk, sb_bk, sb_kT, 0),
                        (sb_wq, sb_bq, sb_qT, 0),
                        (sb_wq, sb_bq, sb_qT, 1),
                    ]
                    fps = [
                        ps_big.tile([128, W], f32, tag="big", name=f"fp{i}")
                        for i in range(3)
                    ]
                    for k in range(KE):
                        for i, (sb_w, _sb_b, _dst, c) in enumerate(first):
                            nc.tensor.matmul(
                                fps[i][:, :CH],
                                sb_w[:, k, :],
                                sb_qkvT[:, k, c * CH : (c + 1) * CH],
                                start=(k == 0),
                                stop=(k == KE - 1),
                            )
                    for i, (_sb_w, sb_b, sb_dst, c) in enumerate(first):
                        nc.vector.tensor_scalar_add(
                            sb_dst[:, c * CH : (c + 1) * CH], fps[i][:, :CH], sb_b[:]
                        )
                    # kT chunk g feeds score t-tiles 4g..4g+3; interleave
                    # v-proj and the remaining q chunks as ungated PE work
                    def proj_qk_part(sb_w, sb_b, sb_dst, c, sb_qkvT, ps, ks):
                        for k in ks:
                            nc.tensor.matmul(
                                ps[:, :CH],
                                sb_w[:, k, :],
                                sb_qkvT[:, k, c * CH : (c + 1) * CH],
                                start=(k == 0),
                                stop=(k == KE - 1),
                            )
                        if ks[-1] == KE - 1:
                            nc.vector.tensor_scalar_add(
                                sb_dst[:, c * CH : (c + 1) * CH],
                                ps[:, :CH],
                                sb_b[:],
                            )

                    kps = None
                    for g in range(4):
                        for t in range(4 * g, 4 * g + 4):
                            # next group's kT chunk: 2 contraction steps per t
                            if g < 3:
                                ti = t - 4 * g
                                if ti == 0:
                                    kps = ps_big.tile(
                                        [128, W], f32, tag="big", name="kps"
                                    )
                                proj_qk_part(
                                    sb_wk, sb_bk, sb_kT, g + 1, sb_qkvT,
                                    kps, [2 * ti, 2 * ti + 1],
                                )
                            scores_t(exA, 0, t)
                            proj_v(t, sb_qkvT)
                            if g == 0 and t == 1:
                                proj_qk(sb_wq, sb_bq, sb_qT, 2, sb_qkvT)
                            if g == 0 and t == 2:
                                proj_qk(sb_wq, sb_bq, sb_qT, 3, sb_qkvT)

                with tc.tile_pool(name="exppB", bufs=1) as exppB:
                    exB = [
                        exppB.tile(
                            [128, NT, W], MM_DT, tag=f"e{h}", name=f"eB{h}"
                        )
                        for h in range(HC)
                    ]
                    # double-rate AV: chunks 0/1 accumulate during the first
                    # half of the P1 score stream, freeing their PSUM slots
                    # for chunks 2/3 in the second half; out_proj runs in a
                    # short tail once ACT is idle
                    avA = av_alloc()
                    for t in range(NT // 2):
                        scores_t(exB, 1, t)
                        av_t(avA, exA, 2 * t)
                        av_t(avA, exA, 2 * t + 1)
                    # per-head order: both chunks of head h first, so tile
                    # avA[h] frees as early as possible for the avB allocs
                    norm_h(avA, 0, 0, 0)
                    norm_h(avA, 0, 1, 0)
                    norm_h(avA, 0, 0, 1)
                    norm_h(avA, 0, 1, 1)
                    avB = av_alloc()
                    for t in range(NT // 2, NT):
                        scores_t(exB, 1, t)
                        j = t - NT // 2
                        if j >= 1:
                            av_t(avB, exB, 2 * (j - 1))
                            av_t(avB, exB, 2 * (j - 1) + 1)
                    av_t(avB, exB, NT - 2)
                    av_t(avB, exB, NT - 1)
                    norm_h(avB, 1, 0, 0)
                    norm_h(avB, 1, 1, 0)
                    norm_h(avB, 1, 0, 1)
                    norm_h(avB, 1, 1, 1)
                    for c in range(NCH):
                        out_proj_chunk(c, act_evict=True)

    nc.finalize()
    return nc


def _prep_inputs(qkv, w_in, b_in, w_out):
    qkv2 = np.asarray(qkv, np.float32).reshape(S, E)
    qkvT = np.ascontiguousarray(qkv2.T).astype(MM_NP)
    w_in = np.asarray(w_in, np.float32)
    b_in = np.asarray(b_in, np.float32)
    w_out = np.asarray(w_out, np.float32)
    in_maps = []
    for c in range(NCORE):
        cols = slice(c * J, c * J + J)
        in_maps.append(
            {
                "qkvT": qkvT,
                "wq": np.ascontiguousarray(w_in[:, :E][:, cols]).astype(MM_NP),
                "wk": np.ascontiguousarray(w_in[:, E : 2 * E][:, cols]).astype(MM_NP),
                "wv": np.ascontiguousarray(w_in[:, 2 * E :][:, cols]).astype(MM_NP),
                "bq": np.ascontiguousarray(b_in[:E][cols]).reshape(J, 1),
                "bk": np.ascontiguousarray(b_in[E : 2 * E][cols]).reshape(J, 1),
                "bv": np.broadcast_to(
                    b_in[2 * E :][cols].reshape(1, J), (128, J)
                ).copy(),
                "wout": np.ascontiguousarray(w_out[cols, :]).astype(MM_NP),
            }
        )
    return in_maps


def kernel(qkv, w_in, b_in, w_out, b_out, _trace=False):
    global _cached
    if _cached is None:
        _cached = _build()
    nc = _cached
    in_maps = _prep_inputs(qkv, w_in, b_in, w_out)
    res = bass_utils.run_bass_kernel_spmd(
        nc, in_maps, core_ids=list(range(NCORE)), trace=_trace
    )
    acc = np.zeros((S, E), np.float64)
    for r in res.results:
        acc += r["partial"].astype(np.float64)
    out = (acc + np.asarray(b_out, np.float32)[None, :]).astype(np.float32)
    out = out.reshape(1, S, E)
    if _trace:
        kernel.last_exec_time_ns = res.exec_time_ns
    return out

